# revision 15
# baseline (speedup 1.0000x reference)
"""CE + CJS loss kernel for Trainium2, data-parallel over 8 NeuronCores.

Math (reference):
    logp = log_softmax(pred_logit, axis=1)          # x - lse_i
    ce   = -mean_i( sum_j gt*logp )
    p    = softmax(pred_logit)
    m    = 0.5*(gt + p + EPS)
    contrib = gt*ln(gt) + p*logp - (gt+p)*ln(m)     # per element
    cjs  = 0.5 * sum_ij w_j * contrib_ij / B,  w_j = C - j
    loss = ce + 0.5*cjs

Kernel decomposition (v4 — bf16 inputs, direct products, 4 PSUM planes):
    xp = x - lse        p = exp(x)/sum      u = gt + p
    a  = gt*ln(gt)      b = u*ln(u/2+eps)   c = p*xp      e3 = gt*xp
    contrib = a + c - b;  CE total = sum_ij e3
Column sums over the batch via TensorE ones-vector matmuls into PSUM:
cs += colsum(a) + colsum(c) - colsum(b) (minus via a -1 stationary),
ce += colsum(e3) superimposed across slices.  The host applies the w_j
weights and assembles the scalar in float64.

Both inputs are cast to bf16 on the host (RNE) before sharding: this
halves HBM traffic, removes on-device f32->bf16 casts, and makes every
DVE operand 2-byte so tensor_scalar runs at 4x and tensor_tensor at 2x.
Engine budget per [128,2048] chunk: ACT runs the three transcendentals
(exp with accum, ln(gt), ln(m)); DVE does xp/p at 4x and the four
products at 2x; PE streams the four accumulation planes.  The Pool
engine is deliberately idle: it shares DVE's SBUF ports and measurably
slows DVE when used.
"""
import os

import numpy as np
from ml_dtypes import bfloat16 as np_bf16

import concourse.bass as bass
import concourse.tile as tile
from concourse import mybir
from concourse.bass_utils import run_bass_kernel_spmd
from concourse.vector_clock import ScopedClock

B, C = 4096, 8192
N_CORES = 8
ROWS = B // N_CORES          # 512 rows per core
N_BLK = ROWS // 128          # 4 partition blocks
F2 = 2048                    # chunk width
N_CHUNK = C // F2            # 4 chunks per block
N_SLICE = C // 512           # 16 matmul column slices
EPS = 1e-8

f32 = mybir.dt.float32
bf16 = mybir.dt.bfloat16
AF = mybir.ActivationFunctionType
ALU = mybir.AluOpType


def _patched_drain_and_barrier(self, tick_clock, wait_clock):
    # Walrus CoreV3 codegen allows only ONE sync-wait command on a
    # Drain/NoOp (NO_STRUCT ctrl). The stock Tile tail drain carries one
    # wait per pending engine clock and fails to compile. Split the waits
    # across single-wait SP nops; SP executes in program order, so the
    # drain still orders after everything.
    nc = self.nc
    probe = nc.sync.nop().ins
    wait_clock.add_sem_waits(probe, ScopedClock({None: tick_clock.global_clock}))
    waits = list(probe.sync_info.on_wait) if probe.sync_info else []
    probe.sync_info = mybir.SyncInfo(on_wait=waits[:1], on_update=[])
    for w in waits[1:]:
        extra = nc.sync.nop().ins
        extra.sync_info = mybir.SyncInfo(on_wait=[w], on_update=[])
    nc.sync.drain()
    nc.all_engine_barrier()
    assert self.sems is not None
    popped = nc._tile_sem_poison_stack.pop()
    assert popped is self._sem_poison
    nc.clear_and_free_semaphores(list(self.sems.allocated().values()))
    nc.all_engine_barrier()


tile.TileContext._drain_and_barrier = _patched_drain_and_barrier


def _split_excess_waits(nc: bass.Bass, max_waits: int = 1):
    # Same walrus limitation, general form: cap sync waits per instruction,
    # hoisting the excess onto same-engine NOPs inserted just before (the
    # engine executes its stream in order, so semantics are unchanged).
    for bb in nc.main_func.blocks:
        insts = list(bb.instructions)
        out, changed = [], False
        for ins in insts:
            si = ins.sync_info
            waits = list(si.on_wait) if (si is not None and si.on_wait) else []
            if len(waits) > max_waits:
                ups = list(si.on_update) if si.on_update else []
                for w in waits[:-max_waits]:
                    nop = mybir.InstNoOp(
                        name=nc.get_next_instruction_name(), ins=[], outs=[])
                    nop.engine = ins.engine
                    nop.sync_info = mybir.SyncInfo(on_wait=[w], on_update=[])
                    nc.register_instruction(nop)
                    out.append(nop)
                ins.sync_info = mybir.SyncInfo(
                    on_wait=waits[-max_waits:], on_update=ups)
                changed = True
            out.append(ins)
        if changed:
            bb.instructions = out


def build_nc() -> bass.Bass:
    nc = bass.Bass()
    x_dram = nc.declare_dram_parameter("x16", [ROWS, C], bf16, isOutput=False)
    gt_dram = nc.declare_dram_parameter("gt16", [ROWS, C], bf16, isOutput=False)
    out_dram = nc.declare_dram_parameter("partials", [N_SLICE, 512], f32, isOutput=True)
    ce_dram = nc.declare_dram_parameter("ce_part", [1, 512], f32, isOutput=True)

    from contextlib import ExitStack
    with tile.TileContext(nc) as tc, ExitStack() as es:
        consts = es.enter_context(tc.tile_pool(name="consts", bufs=1))
        xpool = es.enter_context(tc.tile_pool(name="xpool", bufs=2))
        tpool = es.enter_context(tc.tile_pool(name="tpool", bufs=2))
        rowp = es.enter_context(tc.tile_pool(name="rowp", bufs=2))
        # gt chunk tiles are prefetched a whole block ahead: 2 blocks in
        # flight x 4 chunks = 8 live tiles.
        gtp = es.enter_context(tc.tile_pool(name="gtp", bufs=8))
        ck = es.enter_context(tc.tile_pool(name="ck", bufs=2))
        psum = es.enter_context(tc.tile_pool(name="psum", bufs=1, space="PSUM"))

        ones = consts.tile([128, 1], bf16)
        nc.vector.memset(ones, 1.0)
        neg_ones = consts.tile([128, 1], bf16)
        nc.vector.memset(neg_ones, -1.0)
        eps_half = consts.tile([128, 1], f32)
        nc.vector.memset(eps_half, 0.5 * EPS)

        # contrib column-sum accumulators: one [1,512] row per 512-column
        # slice. PE output base partition must be 0/32/64, so pack 3
        # slices per PSUM bank at those bases.
        banks = [psum.tile([128, 512], f32, name=f"csbank{i}", tag=f"csbank{i}")
                 for i in range((N_SLICE + 2) // 3)]
        def cs_ap(m):
            bank, base = banks[m // 3], 32 * (m % 3)
            return bank[base:base + 1, :]
        # CE accumulator: column sums of e3 from ALL slices superimposed
        # into one 512-wide row; the host sums the 512 values.
        ce_psum = psum.tile([1, 512], f32)

        def emit_head(b):
            """x+gt loads, exp, row stats for block b. Returns per-block state."""
            r0 = b * 128
            x16 = xpool.tile([128, C], bf16, tag="x16")
            tb = tpool.tile([128, C], bf16, tag="t")
            s4 = rowp.tile([128, N_CHUNK], f32, tag="s4")
            gts = []
            for i in range(N_CHUNK):
                xsl = slice(i * F2, (i + 1) * F2)
                nc.sync.dma_start(out=x16[:, xsl], in_=x_dram[r0:r0 + 128, xsl])
                nc.scalar.activation(
                    out=tb[:, xsl], in_=x16[:, xsl], func=AF.Exp,
                    accum_out=s4[:, i:i + 1],
                )
                gt16 = gtp.tile([128, F2], bf16, tag="gt16")
                nc.sync.dma_start(
                    out=gt16[:], in_=gt_dram[r0:r0 + 128, i * F2:(i + 1) * F2])
                gts.append(gt16)
            s = rowp.tile([128, 1], f32, tag="s")
            nc.vector.tensor_reduce(
                out=s[:], in_=s4[:], op=ALU.add, axis=mybir.AxisListType.X,
            )
            recip = rowp.tile([128, 1], f32, tag="recip")
            nc.vector.reciprocal(out=recip[:], in_=s[:])
            lse = rowp.tile([128, 1], f32, tag="lse")
            nc.scalar.activation(out=lse[:], in_=s[:], func=AF.Ln)
            return b, x16, tb, gts, recip, lse

        def emit_chunks(state):
            b, x16, tb, gts, recip, lse = state
            for c in range(N_CHUNK):
                j0 = c * F2
                gt16 = gts[c]
                # Combined operand tiles so the four products run as two
                # wide tensor_tensors:
                #   RR = [lngt | xp | logm],  LL = [p | u]
                #   AE = [gt*lngt | gt*xp]  (left = gt16 repeated, stride-0)
                #   CB = [p*xp | u*logm]
                RR = ck.tile([128, 3 * F2], bf16, tag="RR")
                LL = ck.tile([128, 2 * F2], bf16, tag="LL")

                nc.vector.tensor_scalar(
                    out=RR[:, F2:2 * F2], in0=x16[:, j0:j0 + F2],
                    scalar1=lse[:], scalar2=None, op0=ALU.subtract,
                )
                nc.vector.tensor_scalar(
                    out=LL[:, 0:F2], in0=tb[:, j0:j0 + F2],
                    scalar1=recip[:], scalar2=None, op0=ALU.mult,
                )
                nc.vector.tensor_tensor(
                    out=LL[:, F2:2 * F2], in0=gt16[:], in1=LL[:, 0:F2], op=ALU.add)

                nc.scalar.activation(out=RR[:, 0:F2], in_=gt16[:], func=AF.Ln)
                nc.scalar.activation(
                    out=RR[:, 2 * F2:3 * F2], in_=LL[:, F2:2 * F2], func=AF.Ln,
                    scale=0.5, bias=eps_half[:],
                )

                AE = ck.tile([128, 2 * F2], bf16, tag="AE")
                nc.vector.tensor_tensor(
                    out=AE[:, :].rearrange("p (a b) -> p a b", a=2),
                    in0=gt16[:].unsqueeze(1).broadcast_to([128, 2, F2]),
                    in1=RR[:, 0:2 * F2].rearrange("p (a b) -> p a b", a=2),
                    op=ALU.mult,
                )
                CB = ck.tile([128, 2 * F2], bf16, tag="CB")
                nc.vector.tensor_tensor(
                    out=CB[:, :].rearrange("p (a b) -> p a b", a=2),
                    in0=LL[:, :].rearrange("p (a b) -> p a b", a=2),
                    in1=RR[:, F2:3 * F2].rearrange("p (a b) -> p a b", a=2),
                    op=ALU.mult,
                )

                for k in range(F2 // 512):
                    m = (j0 + k * 512) // 512
                    sl = slice(k * 512, (k + 1) * 512)
                    sl2 = slice(F2 + k * 512, F2 + (k + 1) * 512)
                    nc.tensor.matmul(            # a = gt*lngt
                        cs_ap(m), ones[:], AE[:, sl],
                        start=(b == 0), stop=False,
                    )
                    nc.tensor.matmul(            # c = p*xp
                        cs_ap(m), ones[:], CB[:, sl],
                        start=False, stop=False,
                    )
                    nc.tensor.matmul(            # e3 = gt*xp -> CE row
                        ce_psum[:], ones[:], AE[:, sl2],
                        start=(b == 0 and m == 0),
                        stop=(b == N_BLK - 1 and m == N_SLICE - 1),
                    )
                    nc.tensor.matmul(            # -b = -u*logm
                        cs_ap(m), neg_ones[:], CB[:, sl2],
                        start=False, stop=(b == N_BLK - 1),
                    )

        # Software pipeline: block b+1's exp phase is emitted before block
        # b's chunk phase, so the next x DMAs and exps overlap the chunk
        # compute instead of serializing each block.
        prev = None
        for b in range(N_BLK):
            head = emit_head(b)
            if prev is not None:
                emit_chunks(prev)
            prev = head
        emit_chunks(prev)

        # PSUM is not DMA-readable: bounce through SBUF, split across
        # ScalarE and VectorE so the tail drains faster.
        sb_banks = [consts.tile([128, 512], f32, name=f"sb_cs{i}", tag=f"sb_cs{i}")
                    for i in range(len(banks))]
        for i, bank in enumerate(banks):
            if i % 2 == 0:
                nc.scalar.copy(out=sb_banks[i][:], in_=bank[:])
            else:
                nc.vector.tensor_copy(out=sb_banks[i][:], in_=bank[:])
        sb_ce = consts.tile([1, 512], f32)
        nc.scalar.copy(out=sb_ce[:], in_=ce_psum[:])
        for m in range(N_SLICE):
            bank, base = sb_banks[m // 3], 32 * (m % 3)
            nc.sync.dma_start(out=out_dram[m:m + 1, :], in_=bank[base:base + 1, :])
        nc.sync.dma_start(out=ce_dram[:], in_=sb_ce[:])

    _split_excess_waits(nc)
    return nc


_NC_CACHE = None
LAST_EXEC_NS = None


def kernel(pred_logit: np.ndarray, gt: np.ndarray) -> np.ndarray:
    global _NC_CACHE, LAST_EXEC_NS
    if _NC_CACHE is None:
        _NC_CACHE = build_nc()
    nc = _NC_CACHE

    # Host-side input marshalling: shard rows across cores and lay the
    # operands out in bf16 (RNE), halving HBM traffic per core.
    x16 = np.ascontiguousarray(pred_logit, dtype=np.float32).astype(np_bf16)
    gt16 = np.ascontiguousarray(gt, dtype=np.float32).astype(np_bf16)
    in_maps = [
        {
            "x16": x16[c * ROWS:(c + 1) * ROWS],
            "gt16": gt16[c * ROWS:(c + 1) * ROWS],
        }
        for c in range(N_CORES)
    ]
    run_kwargs = {}
    if os.environ.get("BASS_TRACE"):
        run_kwargs["tmpdir"] = os.environ.get("KERNEL_TRACE_DIR") or None
    res = run_bass_kernel_spmd(nc, in_maps, list(range(N_CORES)), **run_kwargs)
    if res.exec_time_ns is not None:
        LAST_EXEC_NS = res.exec_time_ns

    w = (C - np.arange(C)).astype(np.float64)
    cjs_total = 0.0   # sum_ij w_j * contrib
    ce_total = 0.0    # sum_ij gt * logp
    for r in res.results:
        cs = r["partials"].astype(np.float64).reshape(C)
        cjs_total += np.dot(w, cs)
        ce_total += float(r["ce_part"].astype(np.float64).sum())
    loss = -ce_total / B + 0.25 * cjs_total / B
    return np.array(loss, dtype=np.float32)


# revision 17
# speedup vs baseline: 1.0002x; 1.0002x over previous
"""CE + CJS loss kernel for Trainium2, data-parallel over 8 NeuronCores.

Math (reference):
    logp = log_softmax(pred_logit, axis=1)          # x - lse_i
    ce   = -mean_i( sum_j gt*logp )
    p    = softmax(pred_logit)
    m    = 0.5*(gt + p + EPS)
    contrib = gt*ln(gt) + p*logp - (gt+p)*ln(m)     # per element
    cjs  = 0.5 * sum_ij w_j * contrib_ij / B,  w_j = C - j
    loss = ce + 0.5*cjs

Kernel decomposition (v4 — bf16 inputs, direct products, 4 PSUM planes):
    xp = x - lse        p = exp(x)/sum      u = gt + p
    a  = gt*ln(gt)      b = u*ln(u/2+eps)   c = p*xp      e3 = gt*xp
    contrib = a + c - b;  CE total = sum_ij e3
Column sums over the batch via TensorE ones-vector matmuls into PSUM:
cs += colsum(a) + colsum(c) - colsum(b) (minus via a -1 stationary),
ce += colsum(e3) superimposed across slices.  The host applies the w_j
weights and assembles the scalar in float64.

Both inputs are cast to bf16 on the host (RNE) before sharding: this
halves HBM traffic, removes on-device f32->bf16 casts, and makes every
DVE operand 2-byte so tensor_scalar runs at 4x and tensor_tensor at 2x.
Engine budget per [128,2048] chunk: ACT runs the three transcendentals
(exp with accum, ln(gt), ln(m)); DVE does xp/p at 4x and the four
products at 2x; PE streams the four accumulation planes.  The Pool
engine is deliberately idle: it shares DVE's SBUF ports and measurably
slows DVE when used.
"""
import os

import numpy as np
from ml_dtypes import bfloat16 as np_bf16

import concourse.bass as bass
import concourse.tile as tile
from concourse import mybir
from concourse.bass_utils import run_bass_kernel_spmd
from concourse.vector_clock import ScopedClock

B, C = 4096, 8192
N_CORES = 8
ROWS = B // N_CORES          # 512 rows per core
N_BLK = ROWS // 128          # 4 partition blocks
F2 = 2048                    # chunk width
N_CHUNK = C // F2            # 4 chunks per block
N_SLICE = C // 512           # 16 matmul column slices
EPS = 1e-8

f32 = mybir.dt.float32
bf16 = mybir.dt.bfloat16
AF = mybir.ActivationFunctionType
ALU = mybir.AluOpType


def _patched_drain_and_barrier(self, tick_clock, wait_clock):
    # Walrus CoreV3 codegen allows only ONE sync-wait command on a
    # Drain/NoOp (NO_STRUCT ctrl). The stock Tile tail drain carries one
    # wait per pending engine clock and fails to compile. Split the waits
    # across single-wait SP nops; SP executes in program order, so the
    # drain still orders after everything.
    nc = self.nc
    probe = nc.sync.nop().ins
    wait_clock.add_sem_waits(probe, ScopedClock({None: tick_clock.global_clock}))
    waits = list(probe.sync_info.on_wait) if probe.sync_info else []
    probe.sync_info = mybir.SyncInfo(on_wait=waits[:1], on_update=[])
    for w in waits[1:]:
        extra = nc.sync.nop().ins
        extra.sync_info = mybir.SyncInfo(on_wait=[w], on_update=[])
    nc.sync.drain()
    nc.all_engine_barrier()
    assert self.sems is not None
    popped = nc._tile_sem_poison_stack.pop()
    assert popped is self._sem_poison
    nc.clear_and_free_semaphores(list(self.sems.allocated().values()))
    nc.all_engine_barrier()


tile.TileContext._drain_and_barrier = _patched_drain_and_barrier


def _split_excess_waits(nc: bass.Bass, max_waits: int = 1):
    # Same walrus limitation, general form: cap sync waits per instruction,
    # hoisting the excess onto same-engine NOPs inserted just before (the
    # engine executes its stream in order, so semantics are unchanged).
    for bb in nc.main_func.blocks:
        insts = list(bb.instructions)
        out, changed = [], False
        for ins in insts:
            si = ins.sync_info
            waits = list(si.on_wait) if (si is not None and si.on_wait) else []
            if len(waits) > max_waits:
                ups = list(si.on_update) if si.on_update else []
                for w in waits[:-max_waits]:
                    nop = mybir.InstNoOp(
                        name=nc.get_next_instruction_name(), ins=[], outs=[])
                    nop.engine = ins.engine
                    nop.sync_info = mybir.SyncInfo(on_wait=[w], on_update=[])
                    nc.register_instruction(nop)
                    out.append(nop)
                ins.sync_info = mybir.SyncInfo(
                    on_wait=waits[-max_waits:], on_update=ups)
                changed = True
            out.append(ins)
        if changed:
            bb.instructions = out


def build_nc() -> bass.Bass:
    nc = bass.Bass()
    x_dram = nc.declare_dram_parameter("x16", [ROWS, C], bf16, isOutput=False)
    gt_dram = nc.declare_dram_parameter("gt16", [ROWS, C], bf16, isOutput=False)
    out_dram = nc.declare_dram_parameter("partials", [N_SLICE, 512], f32, isOutput=True)
    ce_dram = nc.declare_dram_parameter("ce_part", [1, 512], f32, isOutput=True)

    from contextlib import ExitStack
    with tile.TileContext(nc) as tc, ExitStack() as es:
        consts = es.enter_context(tc.tile_pool(name="consts", bufs=1))
        xpool = es.enter_context(tc.tile_pool(name="xpool", bufs=2))
        tpool = es.enter_context(tc.tile_pool(name="tpool", bufs=2))
        rowp = es.enter_context(tc.tile_pool(name="rowp", bufs=2))
        # gt chunk tiles are prefetched a whole block ahead: 2 blocks in
        # flight x 4 chunks = 8 live tiles.
        gtp = es.enter_context(tc.tile_pool(name="gtp", bufs=8))
        ck = es.enter_context(tc.tile_pool(name="ck", bufs=2))
        psum = es.enter_context(tc.tile_pool(name="psum", bufs=1, space="PSUM"))

        ones = consts.tile([128, 1], bf16)
        nc.vector.memset(ones, 1.0)
        neg_ones = consts.tile([128, 1], bf16)
        nc.vector.memset(neg_ones, -1.0)
        eps_half = consts.tile([128, 1], f32)
        nc.vector.memset(eps_half, 0.5 * EPS)

        # contrib column-sum accumulators: one [1,512] row per 512-column
        # slice. PE output base partition must be 0/32/64, so pack 3
        # slices per PSUM bank at those bases.
        banks = [psum.tile([128, 512], f32, name=f"csbank{i}", tag=f"csbank{i}")
                 for i in range((N_SLICE + 2) // 3)]
        def cs_ap(m):
            bank, base = banks[m // 3], 32 * (m % 3)
            return bank[base:base + 1, :]
        # CE accumulator: column sums of e3 from ALL slices superimposed
        # into one 512-wide row; the host sums the 512 values.
        ce_psum = psum.tile([1, 512], f32)

        def emit_head(b):
            """x+gt loads and exp for block b. x DMAs go first: they gate
            the lse barrier; gt is only needed a block later."""
            r0 = b * 128
            x16 = xpool.tile([128, C], bf16, tag="x16")
            tb = tpool.tile([128, C], bf16, tag="t")
            s4 = rowp.tile([128, N_CHUNK], f32, tag="s4")
            for i in range(N_CHUNK):
                xsl = slice(i * F2, (i + 1) * F2)
                nc.sync.dma_start(out=x16[:, xsl], in_=x_dram[r0:r0 + 128, xsl])
                nc.scalar.activation(
                    out=tb[:, xsl], in_=x16[:, xsl], func=AF.Exp,
                    accum_out=s4[:, i:i + 1],
                )
            gts = []
            for i in range(N_CHUNK):
                gt16 = gtp.tile([128, F2], bf16, tag="gt16")
                nc.sync.dma_start(
                    out=gt16[:], in_=gt_dram[r0:r0 + 128, i * F2:(i + 1) * F2])
                gts.append(gt16)
            state = [b, x16, tb, gts, s4, None, None]
            return state

        def emit_stats(state):
            """Row stats (sum, recip, lse). Emitted mid-way through the
            previous block's chunk phase so these in-order ops never stall
            the DVE/ACT streams at block boundaries."""
            _, _, _, _, s4, _, _ = state
            s = rowp.tile([128, 1], f32, tag="s")
            nc.vector.tensor_reduce(
                out=s[:], in_=s4[:], op=ALU.add, axis=mybir.AxisListType.X,
            )
            recip = rowp.tile([128, 1], f32, tag="recip")
            nc.vector.reciprocal(out=recip[:], in_=s[:])
            lse = rowp.tile([128, 1], f32, tag="lse")
            nc.scalar.activation(out=lse[:], in_=s[:], func=AF.Ln)
            state[5] = recip
            state[6] = lse

        def emit_chunks(state, after_chunk0=None):
            b, x16, tb, gts, _, recip, lse = state
            for c in range(N_CHUNK):
                j0 = c * F2
                gt16 = gts[c]
                # Combined operand tiles so the four products run as two
                # wide tensor_tensors:
                #   RR = [lngt | xp | logm],  LL = [p | u]
                #   AE = [gt*lngt | gt*xp]  (left = gt16 repeated, stride-0)
                #   CB = [p*xp | u*logm]
                RR = ck.tile([128, 3 * F2], bf16, tag="RR")
                LL = ck.tile([128, 2 * F2], bf16, tag="LL")

                nc.vector.tensor_scalar(
                    out=RR[:, F2:2 * F2], in0=x16[:, j0:j0 + F2],
                    scalar1=lse[:], scalar2=None, op0=ALU.subtract,
                )
                nc.vector.tensor_scalar(
                    out=LL[:, 0:F2], in0=tb[:, j0:j0 + F2],
                    scalar1=recip[:], scalar2=None, op0=ALU.mult,
                )
                nc.vector.tensor_tensor(
                    out=LL[:, F2:2 * F2], in0=gt16[:], in1=LL[:, 0:F2], op=ALU.add)

                nc.scalar.activation(out=RR[:, 0:F2], in_=gt16[:], func=AF.Ln)
                nc.scalar.activation(
                    out=RR[:, 2 * F2:3 * F2], in_=LL[:, F2:2 * F2], func=AF.Ln,
                    scale=0.5, bias=eps_half[:],
                )

                AE = ck.tile([128, 2 * F2], bf16, tag="AE")
                nc.vector.tensor_tensor(
                    out=AE[:, :].rearrange("p (a b) -> p a b", a=2),
                    in0=gt16[:].unsqueeze(1).broadcast_to([128, 2, F2]),
                    in1=RR[:, 0:2 * F2].rearrange("p (a b) -> p a b", a=2),
                    op=ALU.mult,
                )
                CB = ck.tile([128, 2 * F2], bf16, tag="CB")
                nc.vector.tensor_tensor(
                    out=CB[:, :].rearrange("p (a b) -> p a b", a=2),
                    in0=LL[:, :].rearrange("p (a b) -> p a b", a=2),
                    in1=RR[:, F2:3 * F2].rearrange("p (a b) -> p a b", a=2),
                    op=ALU.mult,
                )

                for k in range(F2 // 512):
                    m = (j0 + k * 512) // 512
                    sl = slice(k * 512, (k + 1) * 512)
                    sl2 = slice(F2 + k * 512, F2 + (k + 1) * 512)
                    nc.tensor.matmul(            # a = gt*lngt
                        cs_ap(m), ones[:], AE[:, sl],
                        start=(b == 0), stop=False,
                    )
                    nc.tensor.matmul(            # c = p*xp
                        cs_ap(m), ones[:], CB[:, sl],
                        start=False, stop=False,
                    )
                    nc.tensor.matmul(            # e3 = gt*xp -> CE row
                        ce_psum[:], ones[:], AE[:, sl2],
                        start=(b == 0 and m == 0),
                        stop=(b == N_BLK - 1 and m == N_SLICE - 1),
                    )
                    nc.tensor.matmul(            # -b = -u*logm
                        cs_ap(m), neg_ones[:], CB[:, sl2],
                        start=False, stop=(b == N_BLK - 1),
                    )
                if c == 0 and after_chunk0 is not None:
                    after_chunk0()

        # Software pipeline: block b+1's load/exp phase is emitted before
        # block b's chunk phase, so the next x DMAs and exps overlap the
        # chunk compute instead of serializing each block. Block b+1's row
        # stats are deferred into block b's chunk phase.
        prev = None
        for b in range(N_BLK):
            head = emit_head(b)
            if prev is None:
                emit_stats(head)
            else:
                emit_chunks(prev, after_chunk0=lambda h=head: emit_stats(h))
            prev = head
        emit_chunks(prev)

        # PSUM is not DMA-readable: bounce through SBUF, split across
        # ScalarE and VectorE so the tail drains faster.
        sb_banks = [consts.tile([128, 512], f32, name=f"sb_cs{i}", tag=f"sb_cs{i}")
                    for i in range(len(banks))]
        for i, bank in enumerate(banks):
            if i % 2 == 0:
                nc.scalar.copy(out=sb_banks[i][:], in_=bank[:])
            else:
                nc.vector.tensor_copy(out=sb_banks[i][:], in_=bank[:])
        sb_ce = consts.tile([1, 512], f32)
        nc.scalar.copy(out=sb_ce[:], in_=ce_psum[:])
        for m in range(N_SLICE):
            bank, base = sb_banks[m // 3], 32 * (m % 3)
            nc.sync.dma_start(out=out_dram[m:m + 1, :], in_=bank[base:base + 1, :])
        nc.sync.dma_start(out=ce_dram[:], in_=sb_ce[:])

    _split_excess_waits(nc)
    return nc


_NC_CACHE = None
LAST_EXEC_NS = None


def kernel(pred_logit: np.ndarray, gt: np.ndarray) -> np.ndarray:
    global _NC_CACHE, LAST_EXEC_NS
    if _NC_CACHE is None:
        _NC_CACHE = build_nc()
    nc = _NC_CACHE

    # Host-side input marshalling: shard rows across cores and lay the
    # operands out in bf16 (RNE), halving HBM traffic per core.
    x16 = np.ascontiguousarray(pred_logit, dtype=np.float32).astype(np_bf16)
    gt16 = np.ascontiguousarray(gt, dtype=np.float32).astype(np_bf16)
    in_maps = [
        {
            "x16": x16[c * ROWS:(c + 1) * ROWS],
            "gt16": gt16[c * ROWS:(c + 1) * ROWS],
        }
        for c in range(N_CORES)
    ]
    run_kwargs = {}
    if os.environ.get("BASS_TRACE"):
        run_kwargs["tmpdir"] = os.environ.get("KERNEL_TRACE_DIR") or None
    res = run_bass_kernel_spmd(nc, in_maps, list(range(N_CORES)), **run_kwargs)
    if res.exec_time_ns is not None:
        LAST_EXEC_NS = res.exec_time_ns

    w = (C - np.arange(C)).astype(np.float64)
    cjs_total = 0.0   # sum_ij w_j * contrib
    ce_total = 0.0    # sum_ij gt * logp
    for r in res.results:
        cs = r["partials"].astype(np.float64).reshape(C)
        cjs_total += np.dot(w, cs)
        ce_total += float(r["ce_part"].astype(np.float64).sum())
    loss = -ce_total / B + 0.25 * cjs_total / B
    return np.array(loss, dtype=np.float32)
